# revision 1
# baseline (speedup 1.0000x reference)
"""Trainium2 Bass kernel for nn_MessageLayer (GNN message passing).

Reference computation (per edge, E=1.6M, H=16, DE=32):
    A = (e @ W1 + b1).reshape(E, 16, 16)
    m[e,i] = sum_j A[e,i,j] * h[e,j]  +  (e @ W2 + b2)[e,i]

Strategy (pure data-parallel over E across 8 cores; per core "layout B"):
  partitions = the 256 columns of A^T (two halves of 128, p=(i,j), i major),
  free dim = edges, processed in chunks of F=500 (one PSUM bank), grouped in
  super-chunks of 8 for h-broadcast DMA batching.

  Per super-chunk: load hT(+ones) [17,8F]; replicate its hT rows to all 8
  16-partition groups of sh [128,8F] with 8 on-chip DMAs.
  Per chunk:
    PE:  pa/pb [128,F] = W1half.T @ eT          (A^T without b1)
    DVE: ta/tb [128,F] fp16 = pa/pb * sh-slice  (the irreducible elementwise
         multiply of the einsum; PSUM x SBUF, 1x fp32)
    PE into pm [16,F] (PSUM accumulation):
         W2.T @ eT  +  [B1r;b2].T @ [hT;ones]   (e@W2 + b2 + sum_j b1[i,j] h_j)
       + Ga.T @ ta  +  Gb.T @ tb                (group-sum over j = the reduce)
    ACT: mo = copy(pm)  (DMA cannot read PSUM);  DMA mo -> mT [16,Ec] HBM.
  Host transposes mT back.

All matmuls use fp16 operands (fp32 matmul is 4x slower on the PE; fp16 keeps
input rounding at 2^-11). PSUM accumulation is fp32. Measured rel-l2 error vs
the fp32 reference ~4e-4.
"""

import numpy as np

import concourse.bass as bass
import concourse.mybir as mybir
import concourse.tile as tile
from concourse import bacc
from concourse.bass_utils import run_bass_kernel_spmd

H = 16
DE = 32
NCORES = 8
F = 500  # edges per chunk (matmul free dim; 500*4B = 2000B <= one PSUM bank)
SC = 8  # chunks per super-chunk (h-broadcast batching)

f16 = mybir.dt.float16
f32 = mybir.dt.float32


def build_program(n_super: int):
    """SPMD Bass program for one core processing n_super*SC*F edges."""
    n_chunks = n_super * SC
    Ec = n_chunks * F
    nc = bacc.Bacc("TRN2", target_bir_lowering=False, debug=False)

    eT_d = nc.dram_tensor("eT", [DE, Ec], f16, kind="ExternalInput")
    hT1_d = nc.dram_tensor("hT1", [H + 1, Ec], f16, kind="ExternalInput")
    wa_d = nc.dram_tensor("wa", [DE, 128], f16, kind="ExternalInput")
    wb_d = nc.dram_tensor("wb", [DE, 128], f16, kind="ExternalInput")
    wcomb_d = nc.dram_tensor("wcomb", [DE + H + 1, H], f16, kind="ExternalInput")
    ga_d = nc.dram_tensor("ga", [128, H], f16, kind="ExternalInput")
    gb_d = nc.dram_tensor("gb", [128, H], f16, kind="ExternalInput")
    mT_d = nc.dram_tensor("mT", [H, Ec], f32, kind="ExternalOutput")

    mul = mybir.AluOpType.mult

    with tile.TileContext(nc) as tc:
        with (
            tc.tile_pool(name="const", bufs=1) as cpool,
            tc.tile_pool(name="io", bufs=8) as iopool,
            tc.tile_pool(name="hs", bufs=2) as hspool,
            tc.tile_pool(name="work", bufs=4) as wpool,
            tc.tile_pool(name="psa", bufs=3, space="PSUM") as psa,
            tc.tile_pool(name="psb", bufs=3, space="PSUM") as psb,
            tc.tile_pool(name="psm", bufs=2, space="PSUM") as psm,
        ):
            wa_s = cpool.tile([DE, 128], f16, tag="wa")
            wb_s = cpool.tile([DE, 128], f16, tag="wb")
            wcomb_s = cpool.tile([DE + H + 1, H], f16, tag="wcomb")
            ga_s = cpool.tile([128, H], f16, tag="ga")
            gb_s = cpool.tile([128, H], f16, tag="gb")
            nc.scalar.dma_start(wa_s[:], wa_d[:])
            nc.scalar.dma_start(wb_s[:], wb_d[:])
            nc.scalar.dma_start(wcomb_s[:], wcomb_d[:])
            nc.scalar.dma_start(ga_s[:], ga_d[:])
            nc.scalar.dma_start(gb_s[:], gb_d[:])

            for s in range(n_super):
                ssl = slice(s * SC * F, (s + 1) * SC * F)
                hs = hspool.tile([H + 1, SC * F], f16, tag="hs")
                nc.scalar.dma_start(hs[:], hT1_d[:, ssl])
                sh = hspool.tile([128, SC * F], f16, tag="sh")
                for k in range(8):
                    nc.scalar.dma_start(sh[16 * k : 16 * (k + 1), :], hs[0:H, :])

                for c in range(SC):
                    cc = s * SC + c
                    sl = slice(cc * F, (cc + 1) * F)
                    csl = slice(c * F, (c + 1) * F)

                    x = iopool.tile([DE + H + 1, F], f16, tag="x")
                    nc.scalar.dma_start(x[0:DE, :], eT_d[:, sl])
                    nc.scalar.dma_start(x[DE : DE + H + 1, :], hT1_d[:, sl])

                    pa = psa.tile([128, F], f32, tag="pa")
                    pb = psb.tile([128, F], f32, tag="pb")
                    pm = psm.tile([H, F], f32, tag="pm")

                    nc.tensor.matmul(pa[:], wa_s[:], x[0:DE, :], start=True, stop=True)
                    nc.tensor.matmul(pb[:], wb_s[:], x[0:DE, :], start=True, stop=True)

                    ta = wpool.tile([128, F], f16, tag="ta")
                    tb = wpool.tile([128, F], f16, tag="tb")
                    nc.vector.tensor_tensor(ta[:], pa[:], sh[:, csl], mul)
                    nc.vector.tensor_tensor(tb[:], pb[:], sh[:, csl], mul)

                    nc.tensor.matmul(pm[:], wcomb_s[:], x[:, :], start=True, stop=False)
                    nc.tensor.matmul(pm[:], ga_s[:], ta[:], start=False, stop=False)
                    nc.tensor.matmul(pm[:], gb_s[:], tb[:], start=False, stop=True)

                    mo = wpool.tile([H, F], f32, tag="mo")
                    nc.scalar.copy(mo[:], pm[:])
                    nc.scalar.dma_start(mT_d[:, sl], mo[:])

    nc.compile()
    return nc


def host_prep_weights(W1, b1, W2, b2):
    """Rearrange the dense weights for the device program (fp16)."""
    W1 = np.asarray(W1, np.float32)
    b1 = np.asarray(b1, np.float32)
    W2 = np.asarray(W2, np.float32)
    b2 = np.asarray(b2, np.float32)

    wa = W1[:, :128].astype(np.float16)  # A^T half a: columns (i,j), i<8
    wb = W1[:, 128:].astype(np.float16)  # half b: i>=8

    # Wcomb rows pair with x rows [eT; hT; ones]:
    # W2 [32,16]; B1r[j,i] = b1[i*16+j]; last row = b2
    b1r = b1.reshape(H, H).T
    wcomb = np.concatenate([W2, b1r, b2.reshape(1, H)], axis=0).astype(np.float16)

    # Group indicators: Ga[p, i] = 1 iff p//16 == i ; Gb shifted by 8
    ga = np.zeros((128, H), np.float16)
    gb = np.zeros((128, H), np.float16)
    for p in range(128):
        ga[p, p // H] = 1.0
        gb[p, p // H + 8] = 1.0

    return dict(wa=wa, wb=wb, wcomb=wcomb, ga=ga, gb=gb)


def host_prep_inputs(h, e, Ec_pad):
    """Full [E,*] inputs -> per-core transposed fp16 arrays, padded to Ec_pad."""
    E = e.shape[0]
    per = E // NCORES
    ins = []
    for c in range(NCORES):
        sl = slice(c * per, (c + 1) * per)
        ec = np.zeros((DE, Ec_pad), np.float16)
        ec[:, :per] = e[sl].T.astype(np.float16)
        hc = np.zeros((H + 1, Ec_pad), np.float16)
        hc[:H, :per] = h[sl].T.astype(np.float16)
        hc[H, :] = 1.0
        ins.append((ec, hc))
    return ins


_CACHE = {}


def _get_program(n_super):
    if n_super not in _CACHE:
        _CACHE[n_super] = build_program(n_super)
    return _CACHE[n_super]


def kernel(h, e, W1, b1, W2, b2):
    h = np.asarray(h, np.float32)
    e = np.asarray(e, np.float32)
    E = e.shape[0]
    assert E % NCORES == 0
    per = E // NCORES
    n_super = (per + SC * F - 1) // (SC * F)
    Ec_pad = n_super * SC * F

    nc = _get_program(n_super)
    w = host_prep_weights(W1, b1, W2, b2)
    ins = host_prep_inputs(h, e, Ec_pad)

    in_maps = []
    for c in range(NCORES):
        ec, hc = ins[c]
        in_maps.append(dict(eT=ec, hT1=hc, **w))

    res = run_bass_kernel_spmd(nc, in_maps, core_ids=list(range(NCORES)))

    out = np.empty((E, H), np.float32)
    for c in range(NCORES):
        mT = res.results[c]["mT"]  # [H, Ec_pad] fp32
        out[c * per : (c + 1) * per] = mT[:, :per].T
    return out



# revision 7
# speedup vs baseline: 3.4542x; 3.4542x over previous
"""Trainium2 Bass kernel for nn_MessageLayer (GNN message passing).

Reference computation (per edge, E=1.6M, H=16, DE=32):
    A = (e @ W1 + b1).reshape(E, 16, 16)
    out[e,i] = sum_j A[e,i,j] * h[e,j]  +  (e @ W2 + b2)[e,i]

Pure data-parallel over E across 8 cores. Per core, edges are processed in
blocks of 4096 = 8 chunks x F=512 (one PSUM bank of fp32 per matmul output).

Partition layout ("j-major"): SBUF/PSUM partition p <-> (j, il) with
j = p//8, il = p%8.  Per chunk (512 edges):

  PE:  pa[:,0:512]   = wa^T @ x[0:65]   A^T half a (i=il),    PSUM bank pair
       pa[:,512:1024]= wb^T @ x[0:65]   A^T half b (i=il+8)
  DVE: t = pa * sh   one tensor_tensor over [128,1024]; sh[p] = h[p//8] is
       read twice per chunk via a stride-0 middle AP dim.
  PE into pm [128,512] (24 matmuls per block accumulate, chunk c writes
       partition group 16c..16c+15 via shifted indicator weights):
       wcomb_c^T @ x[0:65]  (e@W2 + b1-term via h rows + b2 via ones row)
     + ga_c^T @ t[:,0:512]  (sum_j over half a)
     + gb_c^T @ t[:,512:]   (sum_j over half b)
  ACT: mo = copy(pm) fp16; one [128,512] copy + one DMA per 4096 edges.

All contraction dims are zero-padded to 65 and output dims to 128 so every
matmul has tile_size (128,128) -- no PE tiling-mode switches. Input DMAs are
issued from the SP queue, the sh broadcast (one DMA, stride-0 replication)
and x-padding memsets from the Pool queue, output from the ACT queue.

fp16 operands on the PE (fp32 matmul is 4x slower); PSUM accumulation fp32;
fp16 output (host upcasts). Measured rel-l2 error vs fp32 reference ~4e-4.
"""

import numpy as np

import concourse.bass as bass
import concourse.mybir as mybir
import concourse.tile as tile
from concourse import bacc
from concourse.ap import AP
from concourse.bass_utils import run_bass_kernel_spmd

H = 16
DE = 32
NCORES = 8
F = 512          # edges per chunk (one PSUM bank of fp32)
BC = 8           # chunks per block
BLK = BC * F     # 4096 edges per block
KP = 65          # padded contraction dim: 32 e + 16 h + 1 ones + 16 zeros

f16 = mybir.dt.float16
f32 = mybir.dt.float32


def _inject_dim(ap, dim):
    """Return a copy of `ap` with `dim` ([stride, size]) inserted after the
    partition dim."""
    dims = [list(d) for d in ap.ap]
    return AP(ap.tensor, ap.offset, [dims[0], dim] + dims[1:])


def build_program(nblk: int):
    """SPMD Bass program for one core processing nblk*BLK edges."""
    Ec = nblk * BLK
    nc = bacc.Bacc("TRN2", target_bir_lowering=False, debug=False)

    eT_d = nc.dram_tensor("eT", [DE, Ec], f16, kind="ExternalInput")
    hT_d = nc.dram_tensor("hT", [H, Ec], f16, kind="ExternalInput")
    wa_d = nc.dram_tensor("wa", [KP, 128], f16, kind="ExternalInput")
    wb_d = nc.dram_tensor("wb", [KP, 128], f16, kind="ExternalInput")
    wcomb_d = nc.dram_tensor("wcomb", [KP, BC * 128], f16, kind="ExternalInput")
    ga_d = nc.dram_tensor("ga", [128, BC * 128], f16, kind="ExternalInput")
    gb_d = nc.dram_tensor("gb", [128, BC * 128], f16, kind="ExternalInput")
    pad_d = nc.dram_tensor("pad", [KP - DE - H, BLK], f16, kind="ExternalInput")
    mT_d = nc.dram_tensor("mT", [128, Ec // BC], f16, kind="ExternalOutput")

    mul = mybir.AluOpType.mult

    XBUFS = 3

    with tile.TileContext(nc) as tc:
        with (
            tc.tile_pool(name="const", bufs=1) as cpool,
            tc.tile_pool(name="sh", bufs=3) as shpool,
            tc.tile_pool(name="t", bufs=3) as tpool,
            tc.tile_pool(name="mo", bufs=2) as mopool,
            tc.tile_pool(name="pa", bufs=3, space="PSUM") as papool,
            tc.tile_pool(name="pm", bufs=2, space="PSUM") as pmpool,
        ):
            wa_s = cpool.tile([KP, 128], f16, tag="wa")
            wb_s = cpool.tile([KP, 128], f16, tag="wb")
            wcomb_s = cpool.tile([KP, BC * 128], f16, tag="wcomb")
            ga_s = cpool.tile([128, BC * 128], f16, tag="ga")
            gb_s = cpool.tile([128, BC * 128], f16, tag="gb")
            nc.scalar.dma_start(wa_s[:], wa_d[:])
            nc.scalar.dma_start(wb_s[:], wb_d[:])
            nc.scalar.dma_start(wcomb_s[:], wcomb_d[:])
            nc.scalar.dma_start(ga_s[:], ga_d[:])
            nc.scalar.dma_start(gb_s[:], gb_d[:])

            # x buffers: one persistent tile, XBUFS manually-rotated block
            # segments. Rows 48 (ones) and 49-64 (zeros) are constant pad --
            # initialized once, never rewritten, so every matmul can take
            # rhs = x[0:65] and run in the uniform (128,128) tile mode.
            x_all = cpool.tile([KP, XBUFS * BLK], f16, tag="x_all")
            for s in range(XBUFS):
                nc.scalar.dma_start(
                    x_all[DE + H : KP, s * BLK : (s + 1) * BLK], pad_d[:]
                )

            for b in range(nblk):
                bsl = slice(b * BLK, (b + 1) * BLK)
                seg = (b % XBUFS) * BLK
                nc.sync.dma_start(x_all[0:DE, seg : seg + BLK], eT_d[:, bsl])
                nc.sync.dma_start(
                    x_all[DE : DE + H, seg : seg + BLK], hT_d[:, bsl]
                )

                # sh[p, f] = h[p // 8, f]: one DMA, 8x replication via a
                # stride-0 free dim on the 16-partition source.
                sh = shpool.tile([128, BLK], f16, tag="sh")
                nc.gpsimd.dma_start(
                    sh[:],
                    _inject_dim(x_all[DE : DE + H, seg : seg + BLK], [0, 8]),
                )

                pm = pmpool.tile([128, F], f32, tag="pm")
                for c in range(BC):
                    xs = x_all[:, seg + c * F : seg + (c + 1) * F]
                    wsl = slice(c * 128, (c + 1) * 128)

                    pa = papool.tile([128, 2 * F], f32, tag="pa")
                    nc.tensor.matmul(
                        pa[:, 0:F], wa_s[:], xs, start=True, stop=True
                    )
                    nc.tensor.matmul(
                        pa[:, F : 2 * F], wb_s[:], xs, start=True, stop=True
                    )

                    t = tpool.tile([128, 2 * F], f16, tag="t")
                    # B operand: sh chunk read twice (stride-0 middle dim)
                    shv = _inject_dim(sh[:, c * F : (c + 1) * F], [0, 2])
                    nc.vector.tensor_tensor(t[:], pa[:], shv, mul)

                    nc.tensor.matmul(
                        pm[:], wcomb_s[:, wsl], xs,
                        start=(c == 0), stop=False,
                    )
                    nc.tensor.matmul(
                        pm[:], ga_s[:, wsl], t[:, 0:F],
                        start=False, stop=False,
                    )
                    nc.tensor.matmul(
                        pm[:], gb_s[:, wsl], t[:, F : 2 * F],
                        start=False, stop=(c == BC - 1),
                    )

                mo = mopool.tile([128, F], f16, tag="mo")
                nc.scalar.copy(mo[:], pm[:])
                nc.scalar.dma_start(mT_d[:, b * F : (b + 1) * F], mo[:])

    nc.compile()
    return nc


def host_prep_weights(W1, b1, W2, b2):
    """Dense weights -> device stationary tensors (fp16, j-major layout)."""
    W1 = np.asarray(W1, np.float32)
    b1 = np.asarray(b1, np.float32)
    W2 = np.asarray(W2, np.float32)
    b2 = np.asarray(b2, np.float32)

    p = np.arange(128)
    jj, il = p // 8, p % 8

    wa = np.zeros((KP, 128), np.float32)
    wb = np.zeros((KP, 128), np.float32)
    wa[:DE, :] = W1[:, il * H + jj]
    wb[:DE, :] = W1[:, (il + 8) * H + jj]

    b1r = b1.reshape(H, H).T  # b1r[j, i] = b1[i*H + j]
    wcomb = np.zeros((KP, BC * 128), np.float32)
    ga = np.zeros((128, BC * 128), np.float32)
    gb = np.zeros((128, BC * 128), np.float32)
    for c in range(BC):
        cols = c * 128 + H * c + np.arange(H)  # q = 16c + i within [c*128 ...]
        # columns q (0..127) of variant c live at wcomb[:, c*128 + q]
        q = H * c + np.arange(H)
        wcomb[0:DE, c * 128 + q] = W2
        wcomb[DE : DE + H, c * 128 + q] = b1r
        wcomb[DE + H, c * 128 + q] = b2
        ga[p, c * 128 + H * c + il] = 1.0
        gb[p, c * 128 + H * c + 8 + il] = 1.0

    pad = np.zeros((KP - DE - H, BLK), np.float16)
    pad[0, :] = 1.0  # ones row for b2 / W2 bias path

    return dict(
        wa=wa.astype(np.float16),
        wb=wb.astype(np.float16),
        wcomb=wcomb.astype(np.float16),
        ga=ga.astype(np.float16),
        gb=gb.astype(np.float16),
        pad=pad,
    )


def host_prep_inputs(h, e, Ec_pad):
    """Full [E,*] inputs -> per-core transposed fp16 arrays, padded."""
    E = e.shape[0]
    per = E // NCORES
    eT = np.zeros((NCORES, DE, Ec_pad), np.float16)
    hT = np.zeros((NCORES, H, Ec_pad), np.float16)
    e3 = np.asarray(e, np.float32).reshape(NCORES, per, DE)
    h3 = np.asarray(h, np.float32).reshape(NCORES, per, H)
    eT[:, :, :per] = e3.transpose(0, 2, 1).astype(np.float16)
    hT[:, :, :per] = h3.transpose(0, 2, 1).astype(np.float16)
    return eT, hT


def unpack_output(mT_all, E):
    """mT per core [128, Ec//8] fp16 -> full [E, H] fp32.

    mT[16c + i, b*F + f] = m(edge b*BLK + c*F + f, i)
    """
    per = E // NCORES
    out = np.empty((E, H), np.float32)
    for core in range(NCORES):
        mT = np.asarray(mT_all[core], np.float32)  # [128, nblk*F]
        nb = mT.shape[1] // F
        m = mT.reshape(BC, H, nb, F).transpose(2, 0, 3, 1).reshape(-1, H)
        out[core * per : (core + 1) * per] = m[:per]
    return out


_CACHE = {}


def _get_program(nblk):
    if nblk not in _CACHE:
        _CACHE[nblk] = build_program(nblk)
    return _CACHE[nblk]


def kernel(h, e, W1, b1, W2, b2):
    e = np.asarray(e)
    E = e.shape[0]
    assert E % NCORES == 0
    per = E // NCORES
    nblk = (per + BLK - 1) // BLK
    Ec_pad = nblk * BLK

    nc = _get_program(nblk)
    w = host_prep_weights(W1, b1, W2, b2)
    eT, hT = host_prep_inputs(h, e, Ec_pad)

    in_maps = [dict(eT=eT[c], hT=hT[c], **w) for c in range(NCORES)]
    res = run_bass_kernel_spmd(nc, in_maps, core_ids=list(range(NCORES)))
    return unpack_output([res.results[c]["mT"] for c in range(NCORES)], E)


# revision 9
# speedup vs baseline: 5.4941x; 1.5906x over previous
"""Trainium2 Bass kernel for nn_MessageLayer (GNN message passing).

Reference computation (per edge, E=1.6M, H=16, DE=32):
    A = (e @ W1 + b1).reshape(E, 16, 16)
    out[e,i] = sum_j A[e,i,j] * h[e,j]  +  (e @ W2 + b2)[e,i]

Pure data-parallel over E across 8 cores. Per core, edges are processed in
blocks of 4096 = 8 chunks x F=512 (one PSUM bank of fp32 per matmul output).

Partition layout ("j-major"): SBUF/PSUM partition p <-> (j, il) with
j = p//8, il = p%8.  Per chunk (512 edges):

  PE:  pa[:,0:512]   = wa^T @ x[0:65]   A^T half a (i=il),    PSUM bank pair
       pa[:,512:1024]= wb^T @ x[0:65]   A^T half b (i=il+8)
  DVE: t = pa * sh   one tensor_tensor over [128,1024]; sh[p] = h[p//8] is
       read twice per chunk via a stride-0 middle AP dim.
  PE into pm [128,512] (24 matmuls per block accumulate, chunk c writes
       partition group 16c..16c+15 via shifted indicator weights):
       wcomb_c^T @ x[0:65]  (e@W2 + b1-term via h rows + b2 via ones row)
     + ga_c^T @ t[:,0:512]  (sum_j over half a)
     + gb_c^T @ t[:,512:]   (sum_j over half b)
  ACT: mo = copy(pm) fp16; one [128,512] copy + one DMA per 4096 edges.

All contraction dims are zero-padded to 65 and output dims to 128 so every
matmul has tile_size (128,128) -- no PE tiling-mode switches. Input DMAs are
issued from the SP queue, the sh broadcast (one DMA, stride-0 replication)
and x-padding memsets from the Pool queue, output from the ACT queue.

fp16 operands on the PE (fp32 matmul is 4x slower); PSUM accumulation fp32;
fp16 output (host upcasts). Measured rel-l2 error vs fp32 reference ~4e-4.
"""

import numpy as np

import concourse.bass as bass
import concourse.mybir as mybir
import concourse.tile as tile
from concourse import bacc
from concourse.ap import AP
from concourse.bass_utils import run_bass_kernel_spmd

H = 16
DE = 32
NCORES = 8
F = 512          # edges per chunk (one PSUM bank of fp32)
BC = 8           # chunks per block
BLK = BC * F     # 4096 edges per block
KP = 65          # padded contraction dim: 32 e + 16 h + 1 ones + 16 zeros

f16 = mybir.dt.float16
f32 = mybir.dt.float32


def _inject_dim(ap, dim):
    """Return a copy of `ap` with `dim` ([stride, size]) inserted after the
    partition dim."""
    dims = [list(d) for d in ap.ap]
    return AP(ap.tensor, ap.offset, [dims[0], dim] + dims[1:])


def build_program(nblk: int):
    """SPMD Bass program for one core processing nblk*BLK edges."""
    Ec = nblk * BLK
    nc = bacc.Bacc("TRN2", target_bir_lowering=False, debug=False)

    eT_d = nc.dram_tensor("eT", [DE, Ec], f16, kind="ExternalInput")
    hT_d = nc.dram_tensor("hT", [H, Ec], f16, kind="ExternalInput")
    wa_d = nc.dram_tensor("wa", [KP, 128], f16, kind="ExternalInput")
    wb_d = nc.dram_tensor("wb", [KP, 128], f16, kind="ExternalInput")
    wcomb_d = nc.dram_tensor("wcomb", [KP, BC * 128], f16, kind="ExternalInput")
    ga_d = nc.dram_tensor("ga", [128, BC * 128], f16, kind="ExternalInput")
    gb_d = nc.dram_tensor("gb", [128, BC * 128], f16, kind="ExternalInput")
    pad_d = nc.dram_tensor("pad", [KP - DE - H, BLK], f16, kind="ExternalInput")
    mT_d = nc.dram_tensor("mT", [128, Ec // BC], f16, kind="ExternalOutput")

    mul = mybir.AluOpType.mult

    XBUFS = 3

    with tile.TileContext(nc) as tc:
        with (
            tc.tile_pool(name="const", bufs=1) as cpool,
            tc.tile_pool(name="sh", bufs=3) as shpool,
            tc.tile_pool(name="t", bufs=3) as tpool,
            tc.tile_pool(name="mo", bufs=2) as mopool,
            tc.tile_pool(name="pa", bufs=3, space="PSUM") as papool,
            tc.tile_pool(name="pm", bufs=2, space="PSUM") as pmpool,
        ):
            wa_s = cpool.tile([KP, 128], f16, tag="wa")
            wb_s = cpool.tile([KP, 128], f16, tag="wb")
            wcomb_s = cpool.tile([KP, BC * 128], f16, tag="wcomb")
            ga_s = cpool.tile([128, BC * 128], f16, tag="ga")
            gb_s = cpool.tile([128, BC * 128], f16, tag="gb")
            nc.scalar.dma_start(wa_s[:], wa_d[:])
            nc.scalar.dma_start(wb_s[:], wb_d[:])
            nc.scalar.dma_start(wcomb_s[:], wcomb_d[:])
            nc.scalar.dma_start(ga_s[:], ga_d[:])
            nc.scalar.dma_start(gb_s[:], gb_d[:])

            # x buffers: one persistent tile, XBUFS manually-rotated block
            # segments. Rows 48 (ones) and 49-64 (zeros) are constant pad --
            # initialized once, never rewritten, so every matmul can take
            # rhs = x[0:65] and run in the uniform (128,128) tile mode.
            x_all = cpool.tile([KP, XBUFS * BLK], f16, tag="x_all")
            for s in range(XBUFS):
                nc.scalar.dma_start(
                    x_all[DE + H : KP, s * BLK : (s + 1) * BLK], pad_d[:]
                )

            for b in range(nblk):
                bsl = slice(b * BLK, (b + 1) * BLK)
                seg = (b % XBUFS) * BLK
                nc.sync.dma_start(x_all[0:DE, seg : seg + BLK], eT_d[:, bsl])
                nc.sync.dma_start(
                    x_all[DE : DE + H, seg : seg + BLK], hT_d[:, bsl]
                )

                # sh[p, f] = h[p // 8, f]: one DMA, 8x replication via a
                # stride-0 free dim on the 16-partition source.
                sh = shpool.tile([128, BLK], f16, tag="sh")
                nc.gpsimd.dma_start(
                    sh[:],
                    _inject_dim(x_all[DE : DE + H, seg : seg + BLK], [0, 8]),
                )

                pm = pmpool.tile([128, F], f32, tag="pm")
                for c in range(BC):
                    xs = x_all[:, seg + c * F : seg + (c + 1) * F]
                    wsl = slice(c * 128, (c + 1) * 128)

                    pa = papool.tile([128, 2 * F], f32, tag="pa")
                    nc.tensor.matmul(
                        pa[:, 0:F], wa_s[:], xs, start=True, stop=True
                    )
                    nc.tensor.matmul(
                        pa[:, F : 2 * F], wb_s[:], xs, start=True, stop=True
                    )

                    t = tpool.tile([128, 2 * F], f16, tag="t")
                    # B operand: sh chunk read twice (stride-0 middle dim)
                    shv = _inject_dim(sh[:, c * F : (c + 1) * F], [0, 2])
                    if c % 2 == 0:
                        # direct: DVE reads PSUM (1x mode)
                        nc.vector.tensor_tensor(t[:], pa[:], shv, mul)
                    else:
                        # offload: ACT evacuates PSUM to fp16, DVE multiplies
                        # SBUF fp16 x fp16 in 2x mode -- balances DVE vs ACT
                        cp = tpool.tile([128, 2 * F], f16, tag="cp")
                        nc.scalar.copy(cp[:], pa[:])
                        nc.vector.tensor_tensor(t[:], cp[:], shv, mul)

                    nc.tensor.matmul(
                        pm[:], wcomb_s[:, wsl], xs,
                        start=(c == 0), stop=False,
                    )
                    nc.tensor.matmul(
                        pm[:], ga_s[:, wsl], t[:, 0:F],
                        start=False, stop=False,
                    )
                    nc.tensor.matmul(
                        pm[:], gb_s[:, wsl], t[:, F : 2 * F],
                        start=False, stop=(c == BC - 1),
                    )

                mo = mopool.tile([128, F], f16, tag="mo")
                nc.scalar.copy(mo[:], pm[:])
                nc.scalar.dma_start(mT_d[:, b * F : (b + 1) * F], mo[:])

    nc.compile()
    return nc


def host_prep_weights(W1, b1, W2, b2):
    """Dense weights -> device stationary tensors (fp16, j-major layout)."""
    W1 = np.asarray(W1, np.float32)
    b1 = np.asarray(b1, np.float32)
    W2 = np.asarray(W2, np.float32)
    b2 = np.asarray(b2, np.float32)

    p = np.arange(128)
    jj, il = p // 8, p % 8

    wa = np.zeros((KP, 128), np.float32)
    wb = np.zeros((KP, 128), np.float32)
    wa[:DE, :] = W1[:, il * H + jj]
    wb[:DE, :] = W1[:, (il + 8) * H + jj]

    b1r = b1.reshape(H, H).T  # b1r[j, i] = b1[i*H + j]
    wcomb = np.zeros((KP, BC * 128), np.float32)
    ga = np.zeros((128, BC * 128), np.float32)
    gb = np.zeros((128, BC * 128), np.float32)
    for c in range(BC):
        # columns q (0..127) of variant c live at wcomb[:, c*128 + q];
        # chunk c writes output partitions q = 16c + i
        q = H * c + np.arange(H)
        wcomb[0:DE, c * 128 + q] = W2
        wcomb[DE : DE + H, c * 128 + q] = b1r
        wcomb[DE + H, c * 128 + q] = b2
        ga[p, c * 128 + H * c + il] = 1.0
        gb[p, c * 128 + H * c + 8 + il] = 1.0

    pad = np.zeros((KP - DE - H, BLK), np.float16)
    pad[0, :] = 1.0  # ones row for b2 / W2 bias path

    return dict(
        wa=wa.astype(np.float16),
        wb=wb.astype(np.float16),
        wcomb=wcomb.astype(np.float16),
        ga=ga.astype(np.float16),
        gb=gb.astype(np.float16),
        pad=pad,
    )


def host_prep_inputs(h, e, Ec_pad):
    """Full [E,*] inputs -> per-core transposed fp16 arrays, padded."""
    E = e.shape[0]
    per = E // NCORES
    eT = np.zeros((NCORES, DE, Ec_pad), np.float16)
    hT = np.zeros((NCORES, H, Ec_pad), np.float16)
    e3 = np.asarray(e, np.float32).reshape(NCORES, per, DE)
    h3 = np.asarray(h, np.float32).reshape(NCORES, per, H)
    eT[:, :, :per] = e3.transpose(0, 2, 1).astype(np.float16)
    hT[:, :, :per] = h3.transpose(0, 2, 1).astype(np.float16)
    return eT, hT


def unpack_output(mT_all, E):
    """mT per core [128, Ec//8] fp16 -> full [E, H] fp32.

    mT[16c + i, b*F + f] = m(edge b*BLK + c*F + f, i)
    """
    per = E // NCORES
    out = np.empty((E, H), np.float32)
    for core in range(NCORES):
        mT = np.asarray(mT_all[core], np.float32)  # [128, nblk*F]
        nb = mT.shape[1] // F
        m = mT.reshape(BC, H, nb, F).transpose(2, 0, 3, 1).reshape(-1, H)
        out[core * per : (core + 1) * per] = m[:per]
    return out


_CACHE = {}


def _get_program(nblk):
    if nblk not in _CACHE:
        _CACHE[nblk] = build_program(nblk)
    return _CACHE[nblk]


def kernel(h, e, W1, b1, W2, b2):
    e = np.asarray(e)
    E = e.shape[0]
    assert E % NCORES == 0
    per = E // NCORES
    nblk = (per + BLK - 1) // BLK
    Ec_pad = nblk * BLK

    nc = _get_program(nblk)
    w = host_prep_weights(W1, b1, W2, b2)
    eT, hT = host_prep_inputs(h, e, Ec_pad)

    in_maps = [dict(eT=eT[c], hT=hT[c], **w) for c in range(NCORES)]
    res = run_bass_kernel_spmd(nc, in_maps, core_ids=list(range(NCORES)))
    return unpack_output([res.results[c]["mT"] for c in range(NCORES)], E)


# revision 12
# speedup vs baseline: 5.5177x; 1.0043x over previous
"""Trainium2 Bass kernel for nn_MessageLayer (GNN message passing).

Reference computation (per edge, E=1.6M, H=16, DE=32):
    A = (e @ W1 + b1).reshape(E, 16, 16)
    out[e,i] = sum_j A[e,i,j] * h[e,j]  +  (e @ W2 + b2)[e,i]

Pure data-parallel over E across 8 cores. Per core, edges are processed in
blocks of 4096 = 8 chunks x F=512 (one PSUM bank of fp32 per matmul output).

Partition layout ("j-major"): SBUF/PSUM partition p <-> (j, il) with
j = p//8, il = p%8.  Per chunk (512 edges):

  PE:  pa[:,0:512]   = wa^T @ x[0:65]   A^T half a (i=il),    PSUM bank pair
       pa[:,512:1024]= wb^T @ x[0:65]   A^T half b (i=il+8)
  t = pa * sh over [128,1024]; sh[p] = h[p//8] is read twice per chunk via a
       stride-0 middle AP dim. 1 of 3 chunks: direct DVE tensor_tensor from
       PSUM (1x); else ACT first evacuates pa to fp16 SBUF so the DVE
       multiply runs in 2x mode -- balances DVE vs ACT busy time.
  PE into pm [128,512] (24 matmuls per block accumulate, chunk c writes
       partition group 16c..16c+15 via shifted indicator weights):
       wcomb_c^T @ x[0:65]  (e@W2 + b1-term via h rows + b2 via ones row)
     + ga_c^T @ t[:,0:512]  (sum_j over half a)
     + gb_c^T @ t[:,512:]   (sum_j over half b)
  ACT: mo = copy(pm) fp16; one [128,512] copy + one DMA per 4096 edges.

All contraction dims are zero-padded to 65 (the pad rows of x are constant,
DMA'd once from a DRAM constant) and output dims to 128 so every matmul has
tile_size (128,128) -- no PE tiling-mode switches. Input DMAs are issued
from the SP queue, the sh broadcast (one DMA, stride-0 replication) from the
Pool queue, output copy + DMA from the ACT queue.

fp16 operands on the PE (fp32 matmul is 4x slower); PSUM accumulation fp32;
fp16 output (host upcasts). Measured rel-l2 error vs fp32 reference ~4e-4.
"""

import numpy as np

import concourse.mybir as mybir
import concourse.tile as tile
from concourse import bacc
from concourse.ap import AP
from concourse.bass_utils import run_bass_kernel_spmd

H = 16
DE = 32
NCORES = 8
F = 512          # edges per chunk (one PSUM bank of fp32)
BC = 8           # chunks per block
BLK = BC * F     # 4096 edges per block
KP = 65          # padded contraction dim: 32 e + 16 h + 1 ones + 16 zeros

f16 = mybir.dt.float16
f32 = mybir.dt.float32


def _inject_dim(ap, dim):
    """Return a copy of `ap` with `dim` ([stride, size]) inserted after the
    partition dim."""
    dims = [list(d) for d in ap.ap]
    return AP(ap.tensor, ap.offset, [dims[0], dim] + dims[1:])


def build_program(nblk: int):
    """SPMD Bass program for one core processing nblk*BLK edges."""
    Ec = nblk * BLK
    nc = bacc.Bacc("TRN2", target_bir_lowering=False, debug=False)

    eT_d = nc.dram_tensor("eT", [DE, Ec], f16, kind="ExternalInput")
    hT_d = nc.dram_tensor("hT", [H, Ec], f16, kind="ExternalInput")
    wa_d = nc.dram_tensor("wa", [KP, 128], f16, kind="ExternalInput")
    wb_d = nc.dram_tensor("wb", [KP, 128], f16, kind="ExternalInput")
    wcomb_d = nc.dram_tensor("wcomb", [KP, BC * 128], f16, kind="ExternalInput")
    ga_d = nc.dram_tensor("ga", [128, BC * 128], f16, kind="ExternalInput")
    gb_d = nc.dram_tensor("gb", [128, BC * 128], f16, kind="ExternalInput")
    pad_d = nc.dram_tensor("pad", [KP - DE - H, BLK], f16, kind="ExternalInput")
    mT_d = nc.dram_tensor("mT", [128, Ec // BC], f16, kind="ExternalOutput")

    mul = mybir.AluOpType.mult

    XBUFS = 3

    with tile.TileContext(nc) as tc:
        with (
            tc.tile_pool(name="const", bufs=1) as cpool,
            tc.tile_pool(name="sh", bufs=3) as shpool,
            tc.tile_pool(name="t", bufs=3) as tpool,
            tc.tile_pool(name="mo", bufs=2) as mopool,
            tc.tile_pool(name="pa", bufs=3, space="PSUM") as papool,
            tc.tile_pool(name="pm", bufs=2, space="PSUM") as pmpool,
        ):
            wa_s = cpool.tile([KP, 128], f16, tag="wa")
            wb_s = cpool.tile([KP, 128], f16, tag="wb")
            wcomb_s = cpool.tile([KP, BC * 128], f16, tag="wcomb")
            ga_s = cpool.tile([128, BC * 128], f16, tag="ga")
            gb_s = cpool.tile([128, BC * 128], f16, tag="gb")
            nc.scalar.dma_start(wa_s[:], wa_d[:])
            nc.scalar.dma_start(wb_s[:], wb_d[:])
            nc.scalar.dma_start(wcomb_s[:], wcomb_d[:])
            nc.scalar.dma_start(ga_s[:], ga_d[:])
            nc.scalar.dma_start(gb_s[:], gb_d[:])

            # x buffers: one persistent tile, XBUFS manually-rotated block
            # segments. Rows 48 (ones) and 49-64 (zeros) are constant pad --
            # initialized once, never rewritten, so every matmul can take
            # rhs = x[0:65] and run in the uniform (128,128) tile mode.
            x_all = cpool.tile([KP, XBUFS * BLK], f16, tag="x_all")
            for s in range(XBUFS):
                nc.scalar.dma_start(
                    x_all[DE + H : KP, s * BLK : (s + 1) * BLK], pad_d[:]
                )

            for b in range(nblk):
                bsl = slice(b * BLK, (b + 1) * BLK)
                seg = (b % XBUFS) * BLK
                nc.sync.dma_start(x_all[0:DE, seg : seg + BLK], eT_d[:, bsl])
                nc.sync.dma_start(
                    x_all[DE : DE + H, seg : seg + BLK], hT_d[:, bsl]
                )

                # sh[p, f] = h[p // 8, f]: one DMA, 8x replication via a
                # stride-0 free dim on the 16-partition source.
                sh = shpool.tile([128, BLK], f16, tag="sh")
                nc.gpsimd.dma_start(
                    sh[:],
                    _inject_dim(x_all[DE : DE + H, seg : seg + BLK], [0, 8]),
                )

                pm = pmpool.tile([128, F], f32, tag="pm")
                for c in range(BC):
                    xs = x_all[:, seg + c * F : seg + (c + 1) * F]
                    wsl = slice(c * 128, (c + 1) * 128)

                    pa = papool.tile([128, 2 * F], f32, tag="pa")
                    nc.tensor.matmul(
                        pa[:, 0:F], wa_s[:], xs, start=True, stop=True
                    )
                    nc.tensor.matmul(
                        pa[:, F : 2 * F], wb_s[:], xs, start=True, stop=True
                    )

                    t = tpool.tile([128, 2 * F], f16, tag="t")
                    # B operand: sh chunk read twice (stride-0 middle dim)
                    shv = _inject_dim(sh[:, c * F : (c + 1) * F], [0, 2])
                    if c % 3 == 0:
                        # direct: DVE reads PSUM (1x mode)
                        nc.vector.tensor_tensor(t[:], pa[:], shv, mul)
                    else:
                        # offload: ACT evacuates PSUM to fp16, DVE multiplies
                        # SBUF fp16 x fp16 in 2x mode -- balances DVE vs ACT
                        cp = tpool.tile([128, 2 * F], f16, tag="cp")
                        nc.scalar.copy(cp[:], pa[:])
                        nc.vector.tensor_tensor(t[:], cp[:], shv, mul)

                    nc.tensor.matmul(
                        pm[:], wcomb_s[:, wsl], xs,
                        start=(c == 0), stop=False,
                    )
                    nc.tensor.matmul(
                        pm[:], ga_s[:, wsl], t[:, 0:F],
                        start=False, stop=False,
                    )
                    nc.tensor.matmul(
                        pm[:], gb_s[:, wsl], t[:, F : 2 * F],
                        start=False, stop=(c == BC - 1),
                    )

                mo = mopool.tile([128, F], f16, tag="mo")
                nc.scalar.copy(mo[:], pm[:])
                nc.scalar.dma_start(mT_d[:, b * F : (b + 1) * F], mo[:])

    nc.compile()
    return nc


def host_prep_weights(W1, b1, W2, b2):
    """Dense weights -> device stationary tensors (fp16, j-major layout)."""
    W1 = np.asarray(W1, np.float32)
    b1 = np.asarray(b1, np.float32)
    W2 = np.asarray(W2, np.float32)
    b2 = np.asarray(b2, np.float32)

    p = np.arange(128)
    jj, il = p // 8, p % 8

    wa = np.zeros((KP, 128), np.float32)
    wb = np.zeros((KP, 128), np.float32)
    wa[:DE, :] = W1[:, il * H + jj]
    wb[:DE, :] = W1[:, (il + 8) * H + jj]

    b1r = b1.reshape(H, H).T  # b1r[j, i] = b1[i*H + j]
    wcomb = np.zeros((KP, BC * 128), np.float32)
    ga = np.zeros((128, BC * 128), np.float32)
    gb = np.zeros((128, BC * 128), np.float32)
    for c in range(BC):
        # columns q (0..127) of variant c live at wcomb[:, c*128 + q];
        # chunk c writes output partitions q = 16c + i
        q = H * c + np.arange(H)
        wcomb[0:DE, c * 128 + q] = W2
        wcomb[DE : DE + H, c * 128 + q] = b1r
        wcomb[DE + H, c * 128 + q] = b2
        ga[p, c * 128 + H * c + il] = 1.0
        gb[p, c * 128 + H * c + 8 + il] = 1.0

    pad = np.zeros((KP - DE - H, BLK), np.float16)
    pad[0, :] = 1.0  # ones row for b2 / W2 bias path

    return dict(
        wa=wa.astype(np.float16),
        wb=wb.astype(np.float16),
        wcomb=wcomb.astype(np.float16),
        ga=ga.astype(np.float16),
        gb=gb.astype(np.float16),
        pad=pad,
    )


def host_prep_inputs(h, e, Ec_pad):
    """Full [E,*] inputs -> per-core transposed fp16 arrays, padded."""
    E = e.shape[0]
    per = E // NCORES
    eT = np.zeros((NCORES, DE, Ec_pad), np.float16)
    hT = np.zeros((NCORES, H, Ec_pad), np.float16)
    e3 = np.asarray(e, np.float32).reshape(NCORES, per, DE)
    h3 = np.asarray(h, np.float32).reshape(NCORES, per, H)
    eT[:, :, :per] = e3.transpose(0, 2, 1).astype(np.float16)
    hT[:, :, :per] = h3.transpose(0, 2, 1).astype(np.float16)
    return eT, hT


def unpack_output(mT_all, E):
    """mT per core [128, Ec//8] fp16 -> full [E, H] fp32.

    mT[16c + i, b*F + f] = m(edge b*BLK + c*F + f, i)
    """
    per = E // NCORES
    out = np.empty((E, H), np.float32)
    for core in range(NCORES):
        mT = np.asarray(mT_all[core], np.float32)  # [128, nblk*F]
        nb = mT.shape[1] // F
        m = mT.reshape(BC, H, nb, F).transpose(2, 0, 3, 1).reshape(-1, H)
        out[core * per : (core + 1) * per] = m[:per]
    return out


_CACHE = {}


def _get_program(nblk):
    if nblk not in _CACHE:
        _CACHE[nblk] = build_program(nblk)
    return _CACHE[nblk]


def kernel(h, e, W1, b1, W2, b2):
    e = np.asarray(e)
    E = e.shape[0]
    assert E % NCORES == 0
    per = E // NCORES
    nblk = (per + BLK - 1) // BLK
    Ec_pad = nblk * BLK

    nc = _get_program(nblk)
    w = host_prep_weights(W1, b1, W2, b2)
    eT, hT = host_prep_inputs(h, e, Ec_pad)

    in_maps = [dict(eT=eT[c], hT=hT[c], **w) for c in range(NCORES)]
    res = run_bass_kernel_spmd(nc, in_maps, core_ids=list(range(NCORES)))
    return unpack_output([res.results[c]["mT"] for c in range(NCORES)], E)


# revision 21
# speedup vs baseline: 5.6900x; 1.0312x over previous
"""Trainium2 Bass kernel for nn_MessageLayer (GNN message passing).

Reference computation (per edge, E=1.6M, H=16, DE=32):
    A = (e @ W1 + b1).reshape(E, 16, 16)
    out[e,i] = sum_j A[e,i,j] * h[e,j]  +  (e @ W2 + b2)[e,i]

Pure data-parallel over E across 8 cores. Per core, edges are processed in
blocks of 4096 = 8 chunks x F=512 (one PSUM bank of fp32 per matmul output).

Partition layout ("j-major"): SBUF/PSUM partition p <-> (j, il) with
j = p//8, il = p%8.  Per chunk (512 edges):

  PE:  pa[:,0:512]   = wa^T @ x[0:65]   A^T half a (i=il),    PSUM bank pair
       pa[:,512:1024]= wb^T @ x[0:65]   A^T half b (i=il+8)
  t = pa * sh over [128,1024]; sh[p] = h[p//8] is read twice per chunk via a
       stride-0 middle AP dim. 1 of 3 chunks: direct DVE tensor_tensor from
       PSUM (1x); else ACT first evacuates pa to fp16 SBUF so the DVE
       multiply runs in 2x mode -- balances DVE vs ACT busy time.
  PE into pm [128,512] (24 matmuls per block accumulate, chunk c writes
       partition group 16c..16c+15 via shifted indicator weights):
       wcomb_c^T @ x[0:65]  (e@W2 + b1-term via h rows + b2 via ones row)
     + ga_c^T @ t[:,0:512]  (sum_j over half a)
     + gb_c^T @ t[:,512:]   (sum_j over half b)
  ACT: mo = copy(pm) fp16; one [128,512] copy + one DMA per 4096 edges.

All contraction dims are zero-padded to 65 (the pad rows of x are constant,
DMA'd once from a DRAM constant) and output dims to 128 so every matmul has
tile_size (128,128) -- no PE tiling-mode switches. Input DMAs are issued
from the SP queue, the sh broadcast (one DMA, stride-0 replication) from the
Pool queue, output copy + DMA from the ACT queue.

fp16 operands on the PE (fp32 matmul is 4x slower); PSUM accumulation fp32;
fp16 output (host upcasts). Measured rel-l2 error vs fp32 reference ~4e-4.
"""

import numpy as np

import concourse.mybir as mybir
import concourse.tile as tile
from concourse import bacc
from concourse.ap import AP
from concourse.bass_utils import run_bass_kernel_spmd

H = 16
DE = 32
NCORES = 8
F = 512          # edges per chunk (one PSUM bank of fp32)
BC = 8           # chunks per block
BLK = BC * F     # 4096 edges per block
KP = 65          # padded contraction dim: 32 e + 16 h + 1 ones + 16 zeros

f16 = mybir.dt.float16
f32 = mybir.dt.float32


def _inject_dim(ap, dim):
    """Return a copy of `ap` with `dim` ([stride, size]) inserted after the
    partition dim."""
    dims = [list(d) for d in ap.ap]
    return AP(ap.tensor, ap.offset, [dims[0], dim] + dims[1:])


def build_program(nblk: int):
    """SPMD Bass program for one core processing nblk*BLK edges."""
    Ec = nblk * BLK
    nc = bacc.Bacc("TRN2", target_bir_lowering=False, debug=False)

    eT_d = nc.dram_tensor("eT", [DE, Ec], f16, kind="ExternalInput")
    hT_d = nc.dram_tensor("hT", [H, Ec], f16, kind="ExternalInput")
    wa_d = nc.dram_tensor("wa", [KP, 128], f16, kind="ExternalInput")
    wb_d = nc.dram_tensor("wb", [KP, 128], f16, kind="ExternalInput")
    wcomb_d = nc.dram_tensor("wcomb", [KP, BC * 128], f16, kind="ExternalInput")
    ga_d = nc.dram_tensor("ga", [128, BC * 128], f16, kind="ExternalInput")
    gb_d = nc.dram_tensor("gb", [128, BC * 128], f16, kind="ExternalInput")
    pad_d = nc.dram_tensor("pad", [KP - DE - H, BLK], f16, kind="ExternalInput")
    mT_d = nc.dram_tensor("mT", [128, Ec // BC], f16, kind="ExternalOutput")

    mul = mybir.AluOpType.mult

    XBUFS = 3

    with tile.TileContext(nc) as tc:
        with (
            tc.tile_pool(name="const", bufs=1) as cpool,
            tc.tile_pool(name="sh", bufs=3) as shpool,
            tc.tile_pool(name="t", bufs=3) as tpool,
            tc.tile_pool(name="mo", bufs=2) as mopool,
            tc.tile_pool(name="pa", bufs=3, space="PSUM") as papool,
            tc.tile_pool(name="pm", bufs=2, space="PSUM") as pmpool,
        ):
            wa_s = cpool.tile([KP, 128], f16, tag="wa")
            wb_s = cpool.tile([KP, 128], f16, tag="wb")
            wcomb_s = cpool.tile([KP, BC * 128], f16, tag="wcomb")
            ga_s = cpool.tile([128, BC * 128], f16, tag="ga")
            gb_s = cpool.tile([128, BC * 128], f16, tag="gb")
            nc.scalar.dma_start(wa_s[:], wa_d[:])
            nc.scalar.dma_start(wb_s[:], wb_d[:])
            nc.scalar.dma_start(wcomb_s[:], wcomb_d[:])
            nc.scalar.dma_start(ga_s[:], ga_d[:])
            nc.scalar.dma_start(gb_s[:], gb_d[:])

            # x buffers: one persistent tile, XBUFS manually-rotated block
            # segments. Rows 48 (ones) and 49-64 (zeros) are constant pad --
            # initialized once, never rewritten, so every matmul can take
            # rhs = x[0:65] and run in the uniform (128,128) tile mode.
            x_all = cpool.tile([KP, XBUFS * BLK], f16, tag="x_all")
            for s in range(XBUFS):
                nc.scalar.dma_start(
                    x_all[DE + H : KP, s * BLK : (s + 1) * BLK], pad_d[:]
                )

            for b in range(nblk):
                bsl = slice(b * BLK, (b + 1) * BLK)
                seg = (b % XBUFS) * BLK
                nc.sync.dma_start(x_all[0:DE, seg : seg + BLK], eT_d[:, bsl])
                nc.sync.dma_start(
                    x_all[DE : DE + H, seg : seg + BLK], hT_d[:, bsl]
                )

                # sh[p, f] = h[p // 8, f]: one DMA, 8x replication via a
                # stride-0 free dim on the 16-partition source.
                sh = shpool.tile([128, BLK], f16, tag="sh")
                nc.gpsimd.dma_start(
                    sh[:],
                    _inject_dim(x_all[DE : DE + H, seg : seg + BLK], [0, 8]),
                )

                pm = pmpool.tile([128, F], f32, tag="pm")

                def reduce_chunk(c, xs, t_ga, t_gb):
                    wsl = slice(c * 128, (c + 1) * 128)
                    nc.tensor.matmul(
                        pm[:], wcomb_s[:, wsl], xs,
                        start=(c == 0), stop=False,
                    )
                    nc.tensor.matmul(
                        pm[:], ga_s[:, wsl], t_ga, start=False, stop=False
                    )
                    nc.tensor.matmul(
                        pm[:], gb_s[:, wsl], t_gb,
                        start=False, stop=(c == BC - 1),
                    )

                for p in range(BC // 2):
                    c0, c1 = 2 * p, 2 * p + 1
                    xs0 = x_all[:, seg + c0 * F : seg + (c0 + 1) * F]
                    xs1 = x_all[:, seg + c1 * F : seg + (c1 + 1) * F]

                    pa0 = papool.tile([128, 2 * F], f32, tag="pa")
                    nc.tensor.matmul(
                        pa0[:, 0:F], wa_s[:], xs0, start=True, stop=True
                    )
                    nc.tensor.matmul(
                        pa0[:, F : 2 * F], wb_s[:], xs0, start=True, stop=True
                    )
                    pa1 = papool.tile([128, 2 * F], f32, tag="pa")
                    nc.tensor.matmul(
                        pa1[:, 0:F], wa_s[:], xs1, start=True, stop=True
                    )
                    nc.tensor.matmul(
                        pa1[:, F : 2 * F], wb_s[:], xs1, start=True, stop=True
                    )

                    if (b * (BC // 2) + p) % 3 == 0:
                        # direct pair: per-chunk DVE TT from PSUM (1x mode);
                        # B operand reads the sh chunk twice (stride-0 dim)
                        t0 = tpool.tile([128, 2 * F], f16, tag="t")
                        shv = _inject_dim(sh[:, c0 * F : (c0 + 1) * F], [0, 2])
                        nc.vector.tensor_tensor(t0[:], pa0[:], shv, mul)
                        t1 = tpool.tile([128, 2 * F], f16, tag="t")
                        shv = _inject_dim(sh[:, c1 * F : (c1 + 1) * F], [0, 2])
                        nc.vector.tensor_tensor(t1[:], pa1[:], shv, mul)
                        reduce_chunk(c0, xs0, t0[:, 0:F], t0[:, F : 2 * F])
                        reduce_chunk(c1, xs1, t1[:, 0:F], t1[:, F : 2 * F])
                    else:
                        # offloaded pair: ACT evacuates both chunks' PSUM with
                        # interleaved destinations -> cp2 = [a0|a1|b0|b1], so
                        # ONE [128,2048] DVE TT in 2x mode covers the pair
                        # (B = sh[c0:c0+2chunks] read twice). Halves the DVE
                        # instruction count on the offload path; pa lifetime
                        # is unchanged (freed by the copy, not the TT).
                        cp2 = tpool.tile([128, 4 * F], f16, tag="cp2")
                        base = cp2[:]
                        bd = [list(d) for d in base.ap]
                        for k, pak in ((0, pa0), (1, pa1)):
                            dst = AP(
                                base.tensor,
                                base.offset + k * F,
                                [bd[0], [2 * F, 2], [1, F]],
                            )
                            nc.scalar.copy(dst, pak[:])
                        t2 = tpool.tile([128, 4 * F], f16, tag="t2")
                        shv = _inject_dim(
                            sh[:, c0 * F : (c0 + 2) * F], [0, 2]
                        )
                        nc.vector.tensor_tensor(t2[:], cp2[:], shv, mul)
                        reduce_chunk(c0, xs0, t2[:, 0:F], t2[:, 2 * F : 3 * F])
                        reduce_chunk(
                            c1, xs1, t2[:, F : 2 * F], t2[:, 3 * F : 4 * F]
                        )

                mo = mopool.tile([128, F], f16, tag="mo")
                nc.scalar.copy(mo[:], pm[:])
                nc.scalar.dma_start(mT_d[:, b * F : (b + 1) * F], mo[:])

    nc.compile()
    return nc


def host_prep_weights(W1, b1, W2, b2):
    """Dense weights -> device stationary tensors (fp16, j-major layout)."""
    W1 = np.asarray(W1, np.float32)
    b1 = np.asarray(b1, np.float32)
    W2 = np.asarray(W2, np.float32)
    b2 = np.asarray(b2, np.float32)

    p = np.arange(128)
    jj, il = p // 8, p % 8

    wa = np.zeros((KP, 128), np.float32)
    wb = np.zeros((KP, 128), np.float32)
    wa[:DE, :] = W1[:, il * H + jj]
    wb[:DE, :] = W1[:, (il + 8) * H + jj]

    b1r = b1.reshape(H, H).T  # b1r[j, i] = b1[i*H + j]
    wcomb = np.zeros((KP, BC * 128), np.float32)
    ga = np.zeros((128, BC * 128), np.float32)
    gb = np.zeros((128, BC * 128), np.float32)
    for c in range(BC):
        # columns q (0..127) of variant c live at wcomb[:, c*128 + q];
        # chunk c writes output partitions q = 16c + i
        q = H * c + np.arange(H)
        wcomb[0:DE, c * 128 + q] = W2
        wcomb[DE : DE + H, c * 128 + q] = b1r
        wcomb[DE + H, c * 128 + q] = b2
        ga[p, c * 128 + H * c + il] = 1.0
        gb[p, c * 128 + H * c + 8 + il] = 1.0

    pad = np.zeros((KP - DE - H, BLK), np.float16)
    pad[0, :] = 1.0  # ones row for b2 / W2 bias path

    return dict(
        wa=wa.astype(np.float16),
        wb=wb.astype(np.float16),
        wcomb=wcomb.astype(np.float16),
        ga=ga.astype(np.float16),
        gb=gb.astype(np.float16),
        pad=pad,
    )


def host_prep_inputs(h, e, Ec_pad):
    """Full [E,*] inputs -> per-core transposed fp16 arrays, padded."""
    E = e.shape[0]
    per = E // NCORES
    eT = np.zeros((NCORES, DE, Ec_pad), np.float16)
    hT = np.zeros((NCORES, H, Ec_pad), np.float16)
    e3 = np.asarray(e, np.float32).reshape(NCORES, per, DE)
    h3 = np.asarray(h, np.float32).reshape(NCORES, per, H)
    eT[:, :, :per] = e3.transpose(0, 2, 1).astype(np.float16)
    hT[:, :, :per] = h3.transpose(0, 2, 1).astype(np.float16)
    return eT, hT


def unpack_output(mT_all, E):
    """mT per core [128, Ec//8] fp16 -> full [E, H] fp32.

    mT[16c + i, b*F + f] = m(edge b*BLK + c*F + f, i)
    """
    per = E // NCORES
    out = np.empty((E, H), np.float32)
    for core in range(NCORES):
        mT = np.asarray(mT_all[core], np.float32)  # [128, nblk*F]
        nb = mT.shape[1] // F
        m = mT.reshape(BC, H, nb, F).transpose(2, 0, 3, 1).reshape(-1, H)
        out[core * per : (core + 1) * per] = m[:per]
    return out


_CACHE = {}


def _get_program(nblk):
    if nblk not in _CACHE:
        _CACHE[nblk] = build_program(nblk)
    return _CACHE[nblk]


def kernel(h, e, W1, b1, W2, b2):
    e = np.asarray(e)
    E = e.shape[0]
    assert E % NCORES == 0
    per = E // NCORES
    nblk = (per + BLK - 1) // BLK
    Ec_pad = nblk * BLK

    nc = _get_program(nblk)
    w = host_prep_weights(W1, b1, W2, b2)
    eT, hT = host_prep_inputs(h, e, Ec_pad)

    in_maps = [dict(eT=eT[c], hT=hT[c], **w) for c in range(NCORES)]
    res = run_bass_kernel_spmd(nc, in_maps, core_ids=list(range(NCORES)))
    return unpack_output([res.results[c]["mT"] for c in range(NCORES)], E)
